# revision 1
# baseline (speedup 1.0000x reference)
"""Trainium2 Bass kernel for nn_ImitationHead (dense_mlp).

Computation (per batch row b of 256):
  h  = mean(z[b], spatial)                # [512] <- z [512,16,16]
  h  = relu-MLP chain 512->512->256->128->64
  goal = [goal_point[b,0,3], goal_point[b,1,3], goal_point_speed[b]]
  GRU (hidden 64, input [x(3); goal(3)]) unrolled 8 steps, each step
  followed by an output MLP 64->4(relu)->4->3 producing dx; x += dx.
  Output: the 8 x values -> [256, 8, 3].

Sharding: pure data parallel, batch 256 -> 8 cores x 32.

On-chip layout is fully "transposed" (features on partitions, batch on
the free axis) so no transposes are ever needed:
  - z shard viewed as [16384, 256]; 16 DMAs of [128p, 2, 4, 256] (1 MiB)
    where partition p holds rows {4p..4p+3} of each 512-row batch block,
    giving 4 KiB contiguous DRAM runs per partition.  The resulting
    channel permutation (chunk j, partition p <-> channel 4p+j) is
    undone by permuting the rows of the layer-1 weight on the host.
  - free-axis reduce per 256-chunk -> hT[channel_p, batch] columns,
    split 3:1 between DVE (tensor_reduce) and ACT (activation accum_out)
    so the reduction keeps pace with the DMA stream.
  - join-MLP matmuls: out_T = (W.T as lhsT).T @ h_T, weights pre-
    transposed on the host; biases applied via the ACT bias operand
    fused with the ReLU.
  - GRU: the four pre-activations (r/z gates, i_n, h_n, and the output
    MLP's first layer) live in PERSISTENT PSUM accumulators.  With
    hh' = hh - d  (d = (1-z)*(hh-n)) every W@hh term updates as
    "psum -= W@d", and the x-recurrence folds through the output MLP:
    gi_x += (W_ihx @ W23.T) @ relu(pd1).  This takes the x/output path
    off the per-step critical chain entirely.
  - biases are folded in as an extra all-ones input row (K+1 matmuls)
    at init; the 4->4 and 4->3 output layers are folded into one 4->3
    matrix on the host (no ReLU between them).
  - mean's 1/256 is folded into the first-layer weights on the host.
  - all small constants travel in one packed [128, 819] DMA.
"""

import numpy as np
from contextlib import ExitStack

N_CORES = 8
B = 256
B_SH = B // N_CORES       # 32 batch rows per core
C = 512                   # channels
S = 256                   # spatial 16*16
HID = 64
T = 8                     # pred_len
ROWS = B_SH * C           # 16384 z rows per core
N_DMA = 16                # z DMAs per core
H_PER = 2                 # batch blocks per z DMA
J = 4                     # 256-chunks per partition per batch block

# packed-constants layout: (name, partitions, cols); column offsets accumulate
_PACK = [
    ("biases", 128, 8),     # jb1 x4, jb2 x2, jb3, jb4
    ("whhbt", 65, 192),     # [W_hh.T; (0...0, b_hh_n)]           (init mms)
    ("wgobt", 4, 192),      # [W_ih[:,3:6].T; (b_rz_sum, b_ih_n)] (init mms)
    ("goalones", 4, B_SH),  # [goal.T; ones]
    ("ow1bt", 65, 4),       # [oW1.T; ob1]                        (init pd1)
    ("whhnbt", 64, 192),    # -W_hh.T                 (incremental updates)
    ("wixobt", 33, 192),    # x-path folded through d1: rows0:4 =
                            #   W23 @ W_ihx.T, row32 = W_ihx @ b23
    ("ow1nbt", 64, 4),      # -oW1.T                  (incremental pd1)
    ("ow23bt", 33, 3),      # rows0:4 = W23, row32 = b23  (output dx)
]
_OFF = {}
_ncol = 0
for _n, _p, _c in _PACK:
    _OFF[_n] = _ncol
    _ncol += _c
PACK_COLS = _ncol

_CACHE: dict = {}


def _build_program():
    import concourse.bacc as bacc
    import concourse.tile as tile
    from concourse import mybir

    f32 = mybir.dt.float32
    AF = mybir.ActivationFunctionType
    AX = mybir.AxisListType
    ALU = mybir.AluOpType

    nc = bacc.Bacc("TRN2", target_bir_lowering=False, debug=False)

    z = nc.dram_tensor("z", [ROWS, S], f32, kind="ExternalInput")
    jw1t = nc.dram_tensor("jw1t", [512, 512], f32, kind="ExternalInput")
    jw2t = nc.dram_tensor("jw2t", [512, 256], f32, kind="ExternalInput")
    jw3t = nc.dram_tensor("jw3t", [256, 128], f32, kind="ExternalInput")
    jw4t = nc.dram_tensor("jw4t", [128, 64], f32, kind="ExternalInput")
    wpack = nc.dram_tensor("wpack", [128, PACK_COLS], f32, kind="ExternalInput")
    out_d = nc.dram_tensor("out", [3 * T, B_SH], f32, kind="ExternalOutput")

    with tile.TileContext(nc) as tc, ExitStack() as ctx:
        consts = ctx.enter_context(tc.tile_pool(name="consts", bufs=1))
        zpool = ctx.enter_context(tc.tile_pool(name="zpool", bufs=3))
        hpool = ctx.enter_context(tc.tile_pool(name="hpool", bufs=1))
        work = ctx.enter_context(tc.tile_pool(name="work", bufs=2))
        xpool = ctx.enter_context(tc.tile_pool(name="xpool", bufs=2))
        psum_mlp = ctx.enter_context(
            tc.tile_pool(name="psum_mlp", bufs=2, space="PSUM"))
        psum_gru = ctx.enter_context(
            tc.tile_pool(name="psum_gru", bufs=1, space="PSUM"))

        # --- z stream: 16 x 1MiB DMAs; reduce each [128, 256] chunk ---
        # Row d*1024 + h*512 + 4p + j -> batch b = 2d+h, channel 4p+j.
        hTs = [hpool.tile([128, B_SH], f32, tag=f"hT{j}", name=f"hT{j}")
               for j in range(J)]
        junk = hpool.tile([128, S], f32)         # ACT accum main out
        z_r = z[:].rearrange("(d h p j) s -> d p h j s", h=H_PER, p=128, j=J)
        for d in range(N_DMA):
            zt = zpool.tile([128, H_PER, J, S], f32, tag="zt")
            nc.sync.dma_start(out=zt, in_=z_r[d])
            for h in range(H_PER):
                b = H_PER * d + h
                for j in range(J):
                    if j < 3:
                        nc.vector.tensor_reduce(
                            out=hTs[j][:, b:b + 1], in_=zt[:, h, j, :],
                            axis=AX.X, op=ALU.add)
                    else:
                        nc.scalar.activation(
                            out=junk, in_=zt[:, h, j, :], func=AF.Copy,
                            accum_out=hTs[j][:, b:b + 1])

        # --- constant loads: queued on the same DMA pipe AFTER the z
        # stream; ordered by when each is first needed (w1 k-chunks for
        # layer 1, then w2, then the GRU pack, then w3/w4).
        w1 = consts.tile([128, 4, 512], f32)
        jw1_r = jw1t[:].rearrange("(k p) m -> k p m", p=128)
        for k in range(4):
            nc.sync.dma_start(out=w1[:, k, :], in_=jw1_r[k])
        w2 = consts.tile([128, 4, 256], f32)
        jw2_r = jw2t[:].rearrange("(k p) m -> k p m", p=128)
        for k in range(0, 4, 2):
            nc.sync.dma_start(out=w2[:, k:k + 2, :], in_=jw2_r[k:k + 2])
        wp = consts.tile([128, PACK_COLS], f32)
        nc.sync.dma_start(out=wp, in_=wpack[:])
        w3 = consts.tile([128, 2, 128], f32)
        nc.sync.dma_start(out=w3, in_=jw3t[:].rearrange("(k p) m -> p k m", p=128))
        w4 = consts.tile([128, 64], f32)
        nc.sync.dma_start(out=w4, in_=jw4t[:])

        bs = wp[0:128, _OFF["biases"]:_OFF["biases"] + 8]
        whh = wp[0:65, _OFF["whhbt"]:_OFF["whhbt"] + 192]
        wgo = wp[0:4, _OFF["wgobt"]:_OFF["wgobt"] + 192]
        gl = wp[0:4, _OFF["goalones"]:_OFF["goalones"] + B_SH]
        ow1 = wp[0:65, _OFF["ow1bt"]:_OFF["ow1bt"] + 4]
        whhn = wp[0:64, _OFF["whhnbt"]:_OFF["whhnbt"] + 192]
        wixo = wp[0:33, _OFF["wixobt"]:_OFF["wixobt"] + 192]
        ow1n = wp[0:64, _OFF["ow1nbt"]:_OFF["ow1nbt"] + 4]
        ow23 = wp[0:33, _OFF["ow23bt"]:_OFF["ow23bt"] + 3]

        # ACT table warmup: sigmoid/tanh tables resident before the tail.
        warm = consts.tile([1, 1], f32)
        nc.vector.memset(warm, 0.0)
        nc.scalar.activation(warm, warm, AF.Sigmoid)
        nc.scalar.activation(warm, warm, AF.Tanh)

        # --- join MLP (transposed): hN_T = relu(W @ h_T + b) ---
        h1 = hpool.tile([128, 4, B_SH], f32)
        for m in range(4):
            pt = psum_mlp.tile([128, B_SH], f32, tag="mlp")
            for k in range(4):
                nc.tensor.matmul(pt, w1[:, k, m * 128:(m + 1) * 128], hTs[k],
                                 start=(k == 0), stop=(k == 3))
            nc.scalar.activation(h1[:, m, :], pt, AF.Relu, bias=bs[:, m:m + 1])
        h2 = hpool.tile([128, 2, B_SH], f32)
        for m in range(2):
            pt = psum_mlp.tile([128, B_SH], f32, tag="mlp")
            for k in range(4):
                nc.tensor.matmul(pt, w2[:, k, m * 128:(m + 1) * 128], h1[:, k, :],
                                 start=(k == 0), stop=(k == 3))
            nc.scalar.activation(h2[:, m, :], pt, AF.Relu, bias=bs[:, 4 + m:5 + m])
        h3 = hpool.tile([128, B_SH], f32)
        pt = psum_mlp.tile([128, B_SH], f32, tag="mlp")
        for k in range(2):
            nc.tensor.matmul(pt, w3[:, k, :], h2[:, k, :],
                             start=(k == 0), stop=(k == 1))
        nc.scalar.activation(h3, pt, AF.Relu, bias=bs[:, 6:7])

        # hhg rows 0:64 = GRU hidden state (in-place across steps), row 64 = 1.
        hhg = hpool.tile([65, B_SH], f32)
        nc.vector.memset(hhg[64:65, :], 1.0)
        pt = psum_mlp.tile([64, B_SH], f32, tag="mlp")
        nc.tensor.matmul(pt, w4, h3, start=True, stop=True)
        nc.scalar.activation(hhg[0:64, :], pt, AF.Relu, bias=bs[0:64, 7:8])

        # d1g: relu(pd1) with ones row at partition 32 (engine-writable);
        # rows 4:32 stay zero so the K=33 matmuls see only d1 + bias.
        d1g = hpool.tile([33, B_SH], f32)
        nc.vector.memset(d1g[0:33, :], 0.0)
        nc.vector.memset(d1g[32:33, :], 1.0)

        # --- GRU: persistent psum accumulators, 8 unrolled steps ---
        prz = psum_gru.tile([128, B_SH], f32, tag="prz")   # r/z pre-act
        pin = psum_gru.tile([64, B_SH], f32, tag="pin")    # i_n pre-act
        phn = psum_gru.tile([64, B_SH], f32, tag="phn")    # h_n pre-act
        pd1 = psum_gru.tile([4, B_SH], f32, tag="pd1")     # oW1@hh+ob1
        ptm = psum_gru.tile([64, B_SH], f32, tag="ptm")    # tanh input
        kw = dict(skip_group_check=True)
        nc.tensor.matmul(prz, wgo[:, 0:128], gl, start=True, stop=False, **kw)
        nc.tensor.matmul(prz, whh[:, 0:128], hhg, start=False, stop=False, **kw)
        nc.tensor.matmul(pin, wgo[:, 128:192], gl, start=True, stop=False, **kw)
        nc.tensor.matmul(phn, whh[:, 128:192], hhg, start=True, stop=False, **kw)
        nc.tensor.matmul(pd1, ow1[0:65, :], hhg, start=True, stop=False, **kw)

        x_prev = None
        for t in range(T):
            last = t == T - 1
            # gate path
            rz = work.tile([128, B_SH], f32, tag="rz")
            nc.scalar.activation(rz, prz, AF.Sigmoid)
            tmp = work.tile([64, B_SH], f32, tag="tmp")
            nc.vector.tensor_mul(tmp, rz[0:64, :], phn)     # r * h_n
            nc.vector.tensor_add(ptm, tmp, pin)             # + i_n -> PSUM
            zc = work.tile([64, B_SH], f32, tag="zc")
            nc.vector.tensor_scalar(
                out=zc, in0=rz[64:128, :], scalar1=-1.0, scalar2=1.0,
                op0=ALU.mult, op1=ALU.add)                  # 1 - z
            n_t = work.tile([64, B_SH], f32, tag="n_t")
            nc.scalar.activation(n_t, ptm, AF.Tanh)
            t1 = work.tile([64, B_SH], f32, tag="t1")
            nc.vector.tensor_sub(t1, hhg[0:64, :], n_t)     # hh - n
            dlt = work.tile([64, B_SH], f32, tag="dlt")
            nc.vector.tensor_mul(dlt, zc, t1)               # d = (1-z)(hh-n)

            # hh' = hh - d; pd1 first (it gates the output path), then
            # the other accumulators.
            nc.tensor.matmul(pd1, ow1n, dlt,
                             start=False, stop=last, **kw)
            nc.vector.tensor_scalar_max(d1g[0:4, :], pd1, 0.0)  # d1(hh')
            if not last:
                nc.tensor.matmul(prz, whhn[:, 0:128], dlt,
                                 start=False, stop=False, **kw)
                nc.tensor.matmul(phn, whhn[:, 128:192], dlt,
                                 start=False, stop=(t == T - 2), **kw)
                nc.vector.tensor_sub(hhg[0:64, :], hhg[0:64, :], dlt)
                # x-recurrence folded through d1g
                nc.tensor.matmul(prz, wixo[:, 0:128], d1g,
                                 start=False, stop=(t == T - 2), **kw)
                nc.tensor.matmul(pin, wixo[:, 128:192], d1g,
                                 start=False, stop=(t == T - 2), **kw)

            # x output (off the critical chain)
            pd3 = psum_gru.tile([3, B_SH], f32, tag="pd3")
            nc.tensor.matmul(pd3, ow23, d1g, start=True, stop=True)
            x_new = xpool.tile([3, B_SH], f32, tag="x")
            if x_prev is None:
                nc.vector.tensor_copy(x_new, pd3)
            else:
                nc.vector.tensor_add(x_new, x_prev, pd3)
            nc.sync.dma_start(out=out_d[3 * t:3 * t + 3, :], in_=x_new)
            x_prev = x_new

    nc.compile()
    return nc


def _get_program():
    if "nc" not in _CACHE:
        _CACHE["nc"] = _build_program()
    return _CACHE["nc"]


def make_in_maps(**inputs) -> list[dict]:
    """Host-side packing + data-parallel sharding -> one in_map per core."""
    f = lambda a: np.ascontiguousarray(np.asarray(a, dtype=np.float32))
    z = f(inputs["z"]).reshape(B, C, S)
    gp = f(inputs["goal_point"])
    gps = f(inputs["goal_point_speed"])
    W_ih, W_hh = f(inputs["W_ih"]), f(inputs["W_hh"])
    b_ih, b_hh = f(inputs["b_ih"]), f(inputs["b_hh"])
    oW1, ob1 = f(inputs["oW1"]), f(inputs["ob1"])
    oW2, ob2 = f(inputs["oW2"]), f(inputs["ob2"])
    oW3, ob3 = f(inputs["oW3"]), f(inputs["ob3"])

    # layer-1 weight: fold the 1/S mean scale and the z-layout channel
    # permutation (chunk j, partition p <-> channel 4p+j).
    jw1t = f(inputs["jW1"]).T * np.float32(1.0 / S)
    perm = (4 * np.arange(128)[None, :] + np.arange(4)[:, None]).reshape(-1)
    jw1t = np.ascontiguousarray(jw1t[perm])
    jw2t = np.ascontiguousarray(f(inputs["jW2"]).T)
    jw3t = np.ascontiguousarray(f(inputs["jW3"]).T)
    jw4t = np.ascontiguousarray(f(inputs["jW4"]).T)

    # bias pack [128, 8]: jb1 (4 cols), jb2 (2), jb3 (1), jb4 (1, rows 0:64)
    biases = np.zeros((128, 8), np.float32)
    biases[:, 0:4] = f(inputs["jb1"]).reshape(4, 128).T
    biases[:, 4:6] = f(inputs["jb2"]).reshape(2, 128).T
    biases[:, 6] = f(inputs["jb3"])
    biases[0:64, 7] = f(inputs["jb4"])

    brow = np.concatenate([b_ih[0:128] + b_hh[0:128], b_ih[128:192]])
    wgobt = np.concatenate([W_ih[:, 3:6].T, brow[None, :]])  # [4, 192]
    brow2 = np.concatenate([np.zeros(128, np.float32), b_hh[128:192]])
    whhbt = np.concatenate([W_hh.T, brow2[None, :]])         # [65, 192]
    whhnbt = -W_hh.T                                         # [64, 192]

    ow1bt = np.concatenate([oW1.T, ob1[None, :]])            # [65, 4]
    ow1nbt = -oW1.T                                          # [64, 4]
    w23 = oW2.T @ oW3.T                                      # [4, 3]
    b23 = ob2 @ oW3.T + ob3                                  # [3]
    ow23bt = np.zeros((33, 3), np.float32)
    ow23bt[0:4] = w23
    ow23bt[32] = b23
    # x-recurrence folded through d1:  W_ihx @ dx = (W23 @ W_ihx.T).T@d1...
    wixobt = np.zeros((33, 192), np.float32)
    wixobt[0:4] = w23 @ W_ih[:, 0:3].T                       # [4, 192]
    wixobt[32] = W_ih[:, 0:3] @ b23                          # [192]

    goalT = np.stack([gp[:, 0, 3], gp[:, 1, 3], gps])        # [3, 256]

    segs = dict(biases=biases, whhbt=whhbt, wgobt=wgobt, ow1bt=ow1bt,
                whhnbt=whhnbt, wixobt=wixobt, ow1nbt=ow1nbt, ow23bt=ow23bt)
    in_maps = []
    for i in range(N_CORES):
        sl = slice(i * B_SH, (i + 1) * B_SH)
        go = np.concatenate(
            [goalT[:, sl], np.ones((1, B_SH), np.float32)])  # [4, 32]
        pack = np.zeros((128, PACK_COLS), np.float32)
        for name, parts, cols in _PACK:
            arr = go if name == "goalones" else segs[name]
            pack[0:parts, _OFF[name]:_OFF[name] + cols] = arr
        in_maps.append(dict(
            z=np.ascontiguousarray(z[sl].reshape(ROWS, S)),
            jw1t=jw1t, jw2t=jw2t, jw3t=jw3t, jw4t=jw4t,
            wpack=pack,
        ))
    return in_maps


def unshard_out(results: list[dict]) -> np.ndarray:
    # per-core out [24, 32]: row 3t+c, col b  ->  [32, 8, 3]
    parts = [r["out"].reshape(T, 3, B_SH).transpose(2, 0, 1) for r in results]
    return np.ascontiguousarray(np.concatenate(parts, axis=0), dtype=np.float32)


def kernel(**inputs) -> np.ndarray:
    from concourse.bass_utils import run_bass_kernel_spmd

    nc = _get_program()
    in_maps = make_in_maps(**inputs)
    res = run_bass_kernel_spmd(nc, in_maps, core_ids=list(range(N_CORES)))
    return unshard_out(res.results)

